# revision 1
# baseline (speedup 1.0000x reference)
"""nn_CrossAttention kernel for 8 Trainium2 NeuronCores.

Sharding: data-parallel over batch B=8, one batch element per core, no
collectives. Per-core layout keeps activations transposed ([feature,
token]) so weight matrices load as the natural stationary operand.
Matmuls run in bf16 (fp32 accumulate) except x@W1 / y@W2, which dominate
the output magnitude and stay fp32. Attention uses keys-on-partition
S^T tiles so softmax denominators come from a ones-matmul accumulated in
PSUM; normalization is applied to the per-head attention outputs via
reciprocal + DMA partition-broadcast + one tensor-tensor multiply.
q1r is injected into the output projection PSUM with an identity matmul.
"""
import sys

sys.path.insert(0, "/opt/trn_rl_repo")

import numpy as np
import ml_dtypes

import concourse.bass as bass
import concourse.tile as tile
from concourse import bacc, mybir, bass2jax

F32 = mybir.dt.float32
BF16 = mybir.dt.bfloat16
EXP = mybir.ActivationFunctionType.Exp
COPY = mybir.ActivationFunctionType.Copy
IDENT = mybir.ActivationFunctionType.Identity

N_CORES = 8
H, D = 8, 64          # heads, head_dim
D2 = 2 * D            # 128
NT = 1024             # tokens
C = 512               # model dim
KB = 8                # key blocks of 128
SCALE = D ** -0.5


def _bcast_ap(t, nparts):
    ap = t[:]
    return bass.AP(tensor=ap.tensor, offset=ap.offset,
                   ap=[[0, nparts]] + [list(p) for p in ap.ap[1:]])


def _build(nc):
    dram = {}
    def din(name, shape, dt):
        dram[name] = nc.dram_tensor(name, shape, dt, kind="ExternalInput").ap()
    din("xT", [84, NT], F32)
    din("yT", [50, NT], F32)
    din("W1", [84, C], F32)
    din("W2", [50, C], F32)
    for n in ("w1k", "w2k", "w1v", "w2v"):
        din(n, [C, 512], BF16)
    din("w1q", [C, 1024], BF16)
    din("w2q", [C, 1024], BF16)
    din("wk2", [D, D2], BF16)
    din("wp1", [1024, C], BF16)
    din("wp2", [C, C], BF16)
    din("bp1", [C], F32)
    din("bp2", [C], F32)
    din("ident", [D2, D2], BF16)
    outT = nc.dram_tensor("outT", [2 * C, NT], F32, kind="ExternalOutput").ap()

    with tile.TileContext(nc) as tc:
        _body(tc, nc, dram, outT)
    return dram, outT


def _body(tc, nc, dram, outT):
    from contextlib import ExitStack
    ctx = ExitStack()
    with ctx:
        wts = ctx.enter_context(tc.tile_pool(name="wts", bufs=1))
        acts = ctx.enter_context(tc.tile_pool(name="acts", bufs=1))

        # ---- load persistent weights ----
        def load(pool, name, shape, dt, src_ap=None):
            t = pool.tile(shape, dt, tag=name)
            nc.sync.dma_start(out=t, in_=dram[name] if src_ap is None else src_ap)
            return t

        w1 = load(wts, "W1", [84, C], F32)
        w2 = load(wts, "W2", [50, C], F32)
        wk2 = wts.tile([D2, D2], BF16, tag="wk2", name="wk2")      # rows 64:128 hold Wk2
        nc.sync.dma_start(out=wk2[D:D2, :], in_=dram["wk2"])
        wp1 = [load(wts, f"wp1_{h}", [D2, C], BF16, dram["wp1"][h * D2:(h + 1) * D2, :]) for h in range(H)]
        wp2 = [load(wts, f"wp2_{h}", [D, C], BF16, dram["wp2"][h * D:(h + 1) * D, :]) for h in range(H)]
        bp1 = wts.tile([128, 4], F32, tag="bp1", name="bp1")
        nc.sync.dma_start(out=bp1, in_=dram["bp1"].rearrange("(j p) -> p j", j=4))
        bp2 = wts.tile([128, 4], F32, tag="bp2", name="bp2")
        nc.sync.dma_start(out=bp2, in_=dram["bp2"].rearrange("(j p) -> p j", j=4))
        ident = load(wts, "ident", [D2, D2], BF16)
        ones = wts.tile([128, 1], BF16, tag="ones", name="ones")
        nc.vector.memset(ones, 1.0)

        # persistent activations
        xc = [acts.tile([128, NT], F32, tag=f"xc{j}", name=f"xc{j}") for j in range(4)]
        yc = [acts.tile([128, NT], F32, tag=f"yc{j}", name=f"yc{j}") for j in range(4)]
        knew = [acts.tile([D2, NT], BF16, tag=f"kn{h}", name=f"kn{h}") for h in range(H)]
        q1p = [acts.tile([D2, NT], BF16, tag=f"q1p{h}", name=f"q1p{h}") for h in range(H)]
        q2p = [acts.tile([D2, NT], BF16, tag=f"q2p{h}", name=f"q2p{h}") for h in range(H)]
        vaug = [acts.tile([128, H, D2 + 1], BF16, tag=f"va{kb}", name=f"va{kb}") for kb in range(KB)]
        o1n = [acts.tile([D2, NT], BF16, tag=f"o1n{h}", name=f"o1n{h}") for h in range(H)]
        o2n = [acts.tile([D, NT], BF16, tag=f"o2n{h}", name=f"o2n{h}") for h in range(H)]

        for kb in range(KB):
            nc.vector.memset(vaug[kb], 1.0)

        # ---- phase A: xc = W1^T @ xT, yc = W2^T @ yT (fp32) ----
        with tc.tile_pool(name="psA", bufs=2, space="PSUM") as psA, \
             tc.tile_pool(name="xb_pool", bufs=1) as xbp:
            xts = load(xbp, "xT", [84, NT], F32)
            yts = load(xbp, "yT", [50, NT], F32)
            xcb = [xbp.tile([128, NT], BF16, tag=f"xcb{j}", name=f"xcb{j}") for j in range(4)]
            ycb = [xbp.tile([128, NT], BF16, tag=f"ycb{j}", name=f"ycb{j}") for j in range(4)]
            for (w, src, dstf, dstb) in ((w1, xts, xc, xcb), (w2, yts, yc, ycb)):
                kdim = w.shape[0]
                for j in range(4):
                    ps = psA.tile([128, NT], F32, tag="psA", name="psA")
                    for nb in range(2):
                        nc.tensor.matmul(ps[:, nb * 512:(nb + 1) * 512],
                                         w[0:kdim, j * 128:(j + 1) * 128],
                                         src[0:kdim, nb * 512:(nb + 1) * 512],
                                         start=True, stop=True)
                    nc.scalar.activation(dstf[j], ps, COPY)
                    nc.vector.tensor_copy(dstb[j], ps)

            # ---- phase B: v tiles (interleaved into vaug) ----
            with tc.tile_pool(name="psB", bufs=2, space="PSUM") as psB:
                wv_tiles = {}
                for wv in ("w1v", "w2v"):
                    tl = []
                    for k in range(4):
                        t = wts.tile([128, 512], BF16, tag=f"{wv}_{k}", name=f"{wv}_{k}")
                        nc.sync.dma_start(out=t, in_=dram[wv][k * 128:(k + 1) * 128, :])
                        tl.append(t)
                    wv_tiles[wv] = tl
                for kb in range(KB):
                    for (wv, srcb, lo) in (("w1v", xcb, 0), ("w2v", ycb, D)):
                        wvt = wv_tiles[wv]
                        ps = psB.tile([128, 512], F32, tag="psB", name="psB")
                        for k in range(4):
                            nc.tensor.matmul(
                                ps, srcb[k][:, kb * 128:(kb + 1) * 128],
                                wvt[k],
                                start=(k == 0), stop=(k == 3))
                        nc.vector.tensor_copy(
                            vaug[kb][:, :, lo:lo + D],
                            ps[:].rearrange("p (h d) -> p h d", h=H))

                # ---- phase C: k1/k2 -> knew ; q1p/q2p folded projections ----
                for (wk, srcb, lo) in (("w1k", xcb, 0), ("w2k", ycb, D)):
                    wkt = []
                    for k in range(4):
                        t = wts.tile([128, 512], BF16, tag=f"{wk}_{k}", name=f"{wk}_{k}")
                        nc.sync.dma_start(out=t, in_=dram[wk][k * 128:(k + 1) * 128, :])
                        wkt.append(t)
                    for p in range(4):
                        ps = psA.tile([128, NT], F32, tag="psA", name="psA")
                        for nb in range(2):
                            for k in range(4):
                                nc.tensor.matmul(
                                    ps[:, nb * 512:(nb + 1) * 512],
                                    wkt[k][:, p * 128:(p + 1) * 128],
                                    srcb[k][:, nb * 512:(nb + 1) * 512],
                                    start=(k == 0), stop=(k == 3))
                        nc.vector.tensor_copy(knew[2 * p][lo:lo + D, :], ps[0:D, :])
                        nc.vector.tensor_copy(knew[2 * p + 1][lo:lo + D, :], ps[D:128, :])

                for (wq, srcb, dst) in (("w1q", xcb, q1p), ("w2q", ycb, q2p)):
                    wqt = []
                    for k in range(4):
                        t = wts.tile([128, 1024], BF16, tag=f"{wq}_{k}", name=f"{wq}_{k}")
                        nc.sync.dma_start(out=t, in_=dram[wq][k * 128:(k + 1) * 128, :])
                        wqt.append(t)
                    for h in range(H):
                        ps = psA.tile([128, NT], F32, tag="psA", name="psA")
                        for nb in range(2):
                            for k in range(4):
                                nc.tensor.matmul(
                                    ps[:, nb * 512:(nb + 1) * 512],
                                    wqt[k][:, h * 128:(h + 1) * 128],
                                    srcb[k][:, nb * 512:(nb + 1) * 512],
                                    start=(k == 0), stop=(k == 3))
                        nc.vector.tensor_copy(dst[h], ps)

        # ---- phase D: attention per head ----
        with tc.tile_pool(name="psS", bufs=2, space="PSUM") as psS, \
             tc.tile_pool(name="psO", bufs=2, space="PSUM") as psO, \
             tc.tile_pool(name="ptmp", bufs=3) as ptmp, \
             tc.tile_pool(name="k2pool", bufs=2) as k2pool, \
             tc.tile_pool(name="rpool", bufs=2) as rpool:
            for h in range(H):
                # k2p projection (base-64 operands)
                k2ps = psS.tile([D2, NT], F32, tag="psS", name="psS")
                for nb in range(2):
                    nc.tensor.matmul(k2ps[:, nb * 512:(nb + 1) * 512],
                                     wk2[D:D2, :],
                                     knew[h][D:D2, nb * 512:(nb + 1) * 512],
                                     start=True, stop=True)
                k2pt = k2pool.tile([D2, NT], BF16, tag="k2p", name="k2p")
                nc.vector.tensor_copy(k2pt, k2ps)

                ops2 = psO.tile([128, NT], F32, tag="psO", name="psO")
                ops1 = psO.tile([128, NT], F32, tag="psO", name="psO")
                for (lhs, qin, vlo, vm, om, ops, opart) in (
                        (knew[h], q1p[h], 0, D2, "o1", ops1, D2),
                        (k2pt, q2p[h], D, D2 + 1 - D, "o2", ops2, D + 1)):
                    for kb in range(KB):
                        sps = psS.tile([128, NT], F32, tag="psS", name="psS")
                        for nb in range(2):
                            nc.tensor.matmul(sps[:, nb * 512:(nb + 1) * 512],
                                             lhs[:, kb * 128:(kb + 1) * 128],
                                             qin[:, nb * 512:(nb + 1) * 512],
                                             start=True, stop=True)
                        pt = ptmp.tile([128, NT], BF16, tag="pt", name="pt")
                        nc.scalar.activation(pt, sps, EXP, scale=SCALE)
                        for nb in range(2):
                            nc.tensor.matmul(ops[0:opart, nb * 512:(nb + 1) * 512],
                                             vaug[kb][:, h, vlo:vlo + vm],
                                             pt[:, nb * 512:(nb + 1) * 512],
                                             start=(kb == 0), stop=(kb == KB - 1))
                        if om == "o1":
                            # r1 rides in unused rows 96 of the attn2 PSUM tile
                            for nb in range(2):
                                nc.tensor.matmul(ops2[96:97, nb * 512:(nb + 1) * 512],
                                                 ones,
                                                 pt[:, nb * 512:(nb + 1) * 512],
                                                 start=(kb == 0), stop=(kb == KB - 1),
                                                 tile_position=(0, 96))
                    # normalize
                    rr = rpool.tile([1, NT], F32, tag="rr", name="rr")
                    if om == "o1":
                        nc.vector.tensor_copy(rr, ops2[96:97, :])
                    else:
                        nc.vector.tensor_copy(rr, ops[D:D + 1, :])
                    nc.vector.reciprocal(rr, rr)
                    rrec = rr
                    nparts = D2 if om == "o1" else D
                    rrb = rpool.tile([128, NT], F32, tag="rrb", name="rrb")
                    nc.gpsimd.partition_broadcast(rrb[0:nparts, :], rrec)
                    if om == "o1":
                        nc.vector.tensor_mul(o1n[h], ops[0:D2, :], rrb[0:D2, :])
                    else:
                        nc.vector.tensor_mul(o2n[h], ops[0:D, :], rrb[0:D, :])

        # ---- phase E: output projections + residuals ----
        with tc.tile_pool(name="psE", bufs=2, space="PSUM") as psE, \
             tc.tile_pool(name="outp", bufs=3) as outp:
            for (wp, on, res, bias, q1off, rowoff, kdim) in (
                    (wp1, o1n, xc, bp1, 0, 0, D2),
                    (wp2, o2n, yc, bp2, 4, C, D)):
                for j in range(4):
                    zps = psE.tile([128, NT], F32, tag="psE", name="psE")
                    for nb in range(2):
                        sl = slice(nb * 512, (nb + 1) * 512)
                        for h in range(H):
                            nc.tensor.matmul(zps[:, sl],
                                             wp[h][0:kdim, j * 128:(j + 1) * 128],
                                             on[h][0:kdim, sl],
                                             start=(h == 0), stop=False)
                        nc.tensor.matmul(zps[:, sl], ident,
                                         q1p[q1off + j][:, sl],
                                         start=False, stop=True)
                    of = outp.tile([128, NT], F32, tag="of", name="of")
                    nc.scalar.activation(of, zps, IDENT, bias=bias[:, j:j + 1])
                    nc.vector.tensor_add(of, of, res[j])
                    nc.sync.dma_start(out=outT[rowoff + j * 128:rowoff + (j + 1) * 128, :], in_=of)


class _Runner:
    def __init__(self):
        import jax
        from jax.sharding import Mesh, PartitionSpec
        from jax.experimental.shard_map import shard_map

        nc = bacc.Bacc("TRN2", target_bir_lowering=False, debug=False,
                       num_devices=N_CORES)
        _build(nc)
        nc.compile()
        self.nc = nc

        bass2jax.install_neuronx_cc_hook()
        part_name = nc.partition_id_tensor.name if nc.partition_id_tensor else None
        in_names, out_names, out_avals, self.zero_shapes = [], [], [], []
        for alloc in nc.m.functions[0].allocations:
            if not isinstance(alloc, mybir.MemoryLocationSet):
                continue
            name = alloc.memorylocations[0].name
            if alloc.kind == "ExternalInput":
                if name != part_name:
                    in_names.append(name)
            elif alloc.kind == "ExternalOutput":
                out_names.append(name)
                shape = tuple(alloc.tensor_shape)
                dtype = mybir.dt.np(alloc.dtype)
                out_avals.append(jax.core.ShapedArray(shape, dtype))
                self.zero_shapes.append((shape, dtype))
        self.in_names, self.out_names, self.out_avals = in_names, out_names, out_avals
        n_params, n_outs = len(in_names), len(out_avals)
        all_names = in_names + out_names + ([part_name] if part_name else [])

        def _bodyfn(*args):
            operands = list(args)
            if part_name:
                operands.append(bass2jax.partition_id_tensor())
            outs = bass2jax._bass_exec_p.bind(
                *operands, out_avals=tuple(out_avals), in_names=tuple(all_names),
                out_names=tuple(out_names), lowering_input_output_aliases=(),
                sim_require_finite=True, sim_require_nnan=True, nc=nc)
            return tuple(outs)

        devices = jax.devices()[:N_CORES]
        mesh = Mesh(np.asarray(devices), ("core",))
        self._fn = jax.jit(
            shard_map(_bodyfn, mesh=mesh,
                      in_specs=(PartitionSpec("core"),) * (n_params + n_outs),
                      out_specs=(PartitionSpec("core"),) * n_outs,
                      check_rep=False),
            donate_argnums=tuple(range(n_params, n_params + n_outs)),
            keep_unused=True)
        self._jax = jax

    def __call__(self, in_maps):
        concat_in = [np.concatenate([m[n] for m in in_maps], axis=0)
                     for n in self.in_names]
        zeros = [np.zeros((N_CORES * s[0], *s[1:]), d) for s, d in self.zero_shapes]
        outs = self._fn(*concat_in, *zeros)
        self._jax.block_until_ready(outs)
        return [
            {n: np.asarray(outs[i]).reshape(N_CORES, *self.out_avals[i].shape)[c]
             for i, n in enumerate(self.out_names)}
            for c in range(N_CORES)
        ]


_RUNNER = None


def _get_runner():
    global _RUNNER
    if _RUNNER is None:
        _RUNNER = _Runner()
    return _RUNNER


def _prep_in_maps(inputs):
    f32 = np.float32
    bf = ml_dtypes.bfloat16
    x = np.asarray(inputs["x"], f32)
    y = np.asarray(inputs["y"], f32)
    Wqkv1 = np.asarray(inputs["Wqkv1"], np.float64)
    Wqkv2 = np.asarray(inputs["Wqkv2"], np.float64)
    Wq1 = np.asarray(inputs["Wq1"], np.float64)
    Wq2 = np.asarray(inputs["Wq2"], np.float64)
    w1q = np.zeros((C, 1024), np.float64)
    w2q = np.zeros((C, 1024), np.float64)
    for h in range(H):
        w1q[:, h * D2:(h + 1) * D2] = Wqkv1[:, h * D:(h + 1) * D] @ Wq1
        w2q[:, h * D2:(h + 1) * D2] = Wqkv2[:, h * D:(h + 1) * D] @ Wq2
    shared = {
        "W1": np.ascontiguousarray(inputs["W1"], f32),
        "W2": np.ascontiguousarray(inputs["W2"], f32),
        "w1k": Wqkv1[:, 512:1024].astype(bf),
        "w2k": Wqkv2[:, 512:1024].astype(bf),
        "w1v": Wqkv1[:, 1024:1536].astype(bf),
        "w2v": Wqkv2[:, 1024:1536].astype(bf),
        "w1q": w1q.astype(bf),
        "w2q": w2q.astype(bf),
        "wk2": np.asarray(inputs["Wk2"]).astype(bf),
        "wp1": np.asarray(inputs["Wp1"]).astype(bf),
        "wp2": np.asarray(inputs["Wp2"]).astype(bf),
        "bp1": np.ascontiguousarray(inputs["bp1"], f32),
        "bp2": np.ascontiguousarray(inputs["bp2"], f32),
        "ident": np.eye(D2, dtype=bf),
    }
    in_maps = []
    for b in range(N_CORES):
        m = dict(shared)
        m["xT"] = np.ascontiguousarray(x[b].T)
        m["yT"] = np.ascontiguousarray(y[b].T)
        in_maps.append(m)
    return in_maps


def kernel(**inputs):
    runner = _get_runner()
    in_maps = _prep_in_maps(inputs)
    results = runner(in_maps)
    out = np.stack([results[b]["outT"].T for b in range(N_CORES)], axis=0)
    return out.astype(np.float32)


if __name__ == "__main__":
    rng = np.random.default_rng(0)
    s = 0.02
    inputs = {
        "x": rng.standard_normal((8, NT, 84), dtype=np.float32),
        "y": rng.standard_normal((8, NT, 50), dtype=np.float32),
        "W1": rng.standard_normal((84, C), dtype=np.float32) * s,
        "W2": rng.standard_normal((50, C), dtype=np.float32) * s,
        "Wqkv1": rng.standard_normal((C, 1536), dtype=np.float32) * s,
        "Wqkv2": rng.standard_normal((C, 1536), dtype=np.float32) * s,
        "Wq1": rng.standard_normal((D, D2), dtype=np.float32) * s,
        "Wq2": rng.standard_normal((D, D2), dtype=np.float32) * s,
        "Wk2": rng.standard_normal((D, D2), dtype=np.float32) * s,
        "Wp1": rng.standard_normal((1024, C), dtype=np.float32) * s,
        "bp1": np.zeros(C, np.float32),
        "Wp2": rng.standard_normal((C, C), dtype=np.float32) * s,
        "bp2": np.zeros(C, np.float32),
    }
    out = kernel(**inputs)
    print("out", out.shape, out.dtype, np.abs(out).max())

